# revision 16
# baseline (speedup 1.0000x reference)
"""Trainium2 Bass kernel for nn_FirstToSpike (spiking NN with sequential scan).

Structure of the computation (discovered by analysis of the reference):
  - X (the time-expanded input) is all zeros except a 50-step window per batch
    at t in [idx_b, idx_b+50) where the dense `input` clip is pasted.
  - b1 == 0, so before the window every batch's h1/h2 state is exactly zero;
    the layer-1 input at window step tau is Q[b, tau] = clip[b] @ W1.T,
    optionally gated by the control spike cs (a {0,1} scalar per batch).
  - The control unit's spike cs(t) is 1 whenever its membrane exceeds 0.5;
    with the given weight statistics it is 1 essentially always.  The device
    kernel assumes gate == 1 for all in-window steps; the host VERIFIES this
    afterwards from the returned spike trains and falls back to an exact
    numpy recomputation if it does not hold.
  - After the window, h1 membranes decay by 10x per step, so given
    max_h ||W1[h,:]||_1 < 45 (host-verified) no h1 spike can occur at
    tau >= 51.  Hence a 51-step aligned scan per batch captures every spike.

Device (8 cores, batch-parallel, 16 batches/core), per core:
  1. Q[h, (tau,b)] = W1.T-chunks @ x-chunks   (PE, fp32 or split-bf16)
  2. 52-step LIF scan (DVE):  h1m = fl(0.1*h1m)*(h1m_prev<=0.5) + Q[tau]
     spikes h1s = h1m > 0.5 recorded for all steps.
  3. DMA out the spike trains H1S [128, 52, 7, 16].

Host: assembles X output (exact paste), runs the (cheap) h2/control/bgt
scans over all 300 steps in fp32 exactly as the reference does, computes
rate = sum2/bgt, and verifies every structural assumption.
"""

import os
import numpy as np

B, T, DIN, HID, NCLS = 128, 300, 2312, 800, 10
DIN_PAD = 2432     # 19 * 128: zero-padded contraction dim (uniform k-chunks)
NCORES, BLOC = 8, 16
TAU_IN = 50        # input window length
W = 51             # aligned scan steps (spikes provably stop after tau=50
                   # given max_h ||W1[h,:]||_1 < 45, host-verified)
KCH = DIN_PAD // 128       # 19 uniform contraction chunks (tail zero-padded)
JCH = (HID + 127) // 128   # 7 output-row chunks (last has 32 rows)
NB = 2             # moving-dim blocks of 400 columns (25 tau x 16 b)
NBW = TAU_IN * BLOC // NB  # 400

DECAY = np.float32(0.1)
THRESH = np.float32(0.5)

# "fp32":   plain fp32 matmuls (walrus lowers each to 2 half-rate passes).
# "split16": x/w split into fp16 hi+lo, 3 passes (hi*hi + hi*lo + lo*hi) at
#            full PE rate; max |Q| error ~3e-6 (fp32-reorder level, verified).
MM_MODE = os.environ.get("K_MM_MODE", "split16")

# tau-blocks of the moving dimension: the scan consumes each block's Q while
# the next block's matmuls run on PE.  512 cols = one full PSUM bank; big
# blocks amortize per-matmul issue overhead, the small last block shortens
# the exposed scan tail.
NBS = [32, 18]

_prog_cache = {}
LAST_RESULTS = None  # BassKernelResults stash for test harness introspection


def _build_program(mm_mode):
    import concourse.bass as bass
    import concourse.tile as tile
    import concourse.mybir as mybir
    from concourse import bacc
    from concourse.bass import ds

    f32 = mybir.dt.float32
    bf16 = mybir.dt.bfloat16
    Alu = mybir.AluOpType

    # Bacc (not plain Bass): its compile() pipeline legalizes multi-wait
    # instructions (move_matmul_waits_to_ldweights / generate_event_semaphores)
    # which walrus codegen requires.
    nc = bacc.Bacc("TRN2")

    if mm_mode == "fp32":
        x_names, w_names, dt_in = ["x0"], ["w0"], f32
        passes = [(0, 0)]
    else:
        f16 = mybir.dt.float16
        x_names, w_names, dt_in = ["xhi", "xlo"], ["whi", "wlo"], f16
        passes = [(0, 0), (0, 1), (1, 0)]  # xhi*whi + xhi*wlo + xlo*whi

    x_dram = [nc.dram_tensor(n, [DIN_PAD, TAU_IN * BLOC], dt_in,
                             kind="ExternalInput") for n in x_names]
    w_dram = [nc.dram_tensor(n, [DIN_PAD, HID], dt_in, kind="ExternalInput")
              for n in w_names]
    u8 = mybir.dt.uint8
    h1s_out = nc.dram_tensor("h1s_out", [128, W, JCH, BLOC], u8,
                             kind="ExternalOutput")

    with tile.TileContext(nc) as tc:
        with (
            tc.tile_pool(name="big", bufs=1) as big,
            tc.tile_pool(name="state", bufs=1) as state,
            tc.tile_pool(name="psum", bufs=1, space="PSUM") as psum,
        ):
            x_sb = [big.tile([128, KCH, TAU_IN * BLOC], dt_in, tag=f"x{i}", name=f"x_sb{i}")
                    for i in range(len(x_dram))]
            w_sb = [big.tile([128, KCH, HID], dt_in, tag=f"w{i}", name=f"w_sb{i}")
                    for i in range(len(w_dram))]
            q_sb = big.tile([128, TAU_IN, JCH, BLOC], f32, tag="q")
            h1s_sb = big.tile([128, W, JCH, BLOC], u8, tag="h1s")

            h1m = state.tile([128, JCH, BLOC], f32, tag="h1m")
            zt = state.tile([128, JCH, BLOC], f32, tag="zt")   # zeros (fp32)
            ztm = state.tile([128, JCH, BLOC], u8, tag="ztm")  # zeros (mask)

            # j == JCH-1 h-rows beyond HID are never written by the matmul
            # copies; zero the whole Q buffer once so the scan sees clean pads.
            nc.gpsimd.memset(q_sb[:], 0.0)
            nc.vector.memset(h1m[:], 0.0)
            nc.vector.memset(zt[:], 0.0)
            nc.vector.memset(ztm[:], 0)

            # stream inputs interleaved per k-chunk, in the order the
            # matmul passes consume them (pass0 needs x[0]+w[0] first)
            loads, seen = [], set()
            for (xi, wi) in passes:
                if ("x", xi) not in seen:
                    seen.add(("x", xi)); loads.append((x_sb[xi], x_dram[xi]))
                if ("w", wi) not in seen:
                    seen.add(("w", wi)); loads.append((w_sb[wi], w_dram[wi]))
            for k in range(KCH):
                for t_sb, t_dram in loads[:2]:
                    if k < 2:
                        # 4-way partition split so the first chunks land on
                        # several DMA queues at once (shorter PE start stall)
                        for q in range(4):
                            nc.sync.dma_start(
                                out=t_sb[32 * q:32 * (q + 1), k, :],
                                in_=t_dram[128 * k + 32 * q:
                                           128 * k + 32 * (q + 1), :])
                    else:
                        nc.sync.dma_start(out=t_sb[:, k, :],
                                          in_=t_dram[128 * k:128 * (k + 1), :])
            for t_sb, t_dram in loads[2:]:
                for k in range(KCH):
                    nc.sync.dma_start(out=t_sb[:, k, :],
                                      in_=t_dram[128 * k:128 * (k + 1), :])

            def scan_step(tau):
                qv = q_sb[:, tau] if tau < TAU_IN else zt[:]
                mask = h1s_sb[:, tau - 1] if tau > 0 else ztm[:]
                # h1m' = fl(fl(0.1*h1m) + Q); where previous step spiked the
                # membrane resets, making h1m' exactly Q (reference rounding).
                nc.vector.scalar_tensor_tensor(
                    out=h1m[:], in0=h1m[:], scalar=0.1, in1=qv,
                    op0=Alu.mult, op1=Alu.add)
                nc.vector.copy_predicated(out=h1m[:], mask=mask, data=qv)
                nc.gpsimd.tensor_scalar(h1s_sb[:, tau], h1m[:], 0.5, None,
                                        Alu.is_gt)

            assert sum(NBS) == TAU_IN
            n_acc = len(passes) * KCH
            lo = 0
            for nb, ntau in enumerate(NBS):
                hi = lo + ntau
                cols = ntau * BLOC
                pss = [psum.tile([128, cols], f32, tag=f"ps{j}",
                                 name=f"ps{j}") for j in range(JCH)]
                for pi, (xi, wi) in enumerate(passes):
                    for j in range(JCH):
                        mj = min(128, HID - 128 * j)
                        for k in range(KCH):
                            nc.tensor.matmul(
                                pss[j][:mj, :],
                                lhsT=w_sb[wi][:, k, ds(128 * j, mj)],
                                rhs=x_sb[xi][:, k, ds(lo * BLOC, cols)],
                                start=(pi == 0 and k == 0),
                                stop=(pi == len(passes) - 1 and k == KCH - 1))
                for j in range(JCH):
                    mj = min(128, HID - 128 * j)
                    # PSUM -> Q, reshaping (tau*16+b) columns into [tau, b]
                    nc.scalar.copy(
                        out=q_sb[:mj, ds(lo, ntau), j, :],
                        in_=pss[j][:mj, :].rearrange("p (t b) -> p t b",
                                                     b=BLOC))
                # consume this block's Q slices while the next block's
                # matmuls run on PE
                for tau in range(lo, hi):
                    scan_step(tau)
                if nb < len(NBS) - 1:
                    nc.sync.dma_start(out=h1s_out[:, lo:hi],
                                      in_=h1s_sb[:, lo:hi])
                lo = hi

            for tau in range(TAU_IN, W):
                scan_step(tau)
            last_lo = TAU_IN - NBS[-1]
            nc.sync.dma_start(out=h1s_out[:, last_lo:],
                              in_=h1s_sb[:, last_lo:])

    nc.finalize()  # Bacc.compile(): wait legalization + register allocation
    return nc


def _get_program(mm_mode):
    if mm_mode not in _prog_cache:
        _prog_cache[mm_mode] = _build_program(mm_mode)
    return _prog_cache[mm_mode]


def _np_exact_reference(inp, X_in, idx, W1, b1, W2, b2, Wc, bc):
    """Exact fp32 numpy replication of the reference (fallback path).

    Uses x_a @ W1.T == gate * (x @ W1.T) (exact: gate in {0,1}) when X has
    the zeros+window structure; otherwise fully dense (slow but correct).
    """
    Bn = X_in.shape[0]
    Xf = X_in.reshape(Bn, DIN, T)
    t_arr = np.arange(T)
    freqs = np.stack([np.ones(T), (t_arr % 2 == 0), (t_arr % 10 == 0),
                      (t_arr % 100 == 0)], axis=1).astype(np.float32)
    # paste
    Xp = Xf.copy()
    clip = inp.reshape(Bn, DIN, TAU_IN)
    for b in range(Bn):
        Xp[b, :, idx[b]:idx[b] + TAU_IN] = clip[b]
    h1m = np.zeros((Bn, HID), np.float32); h1s = np.zeros((Bn, HID), np.float32)
    h2m = np.zeros((Bn, NCLS), np.float32); h2s = np.zeros((Bn, NCLS), np.float32)
    cm = np.zeros((Bn, 1), np.float32); cs = np.zeros((Bn, 1), np.float32)
    bgt = np.ones((Bn, 1), np.float32); sum2 = np.zeros((Bn, NCLS), np.float32)
    for t in range(T):
        x = np.ascontiguousarray(Xp[:, :, t])
        gate = np.where(t == 0, np.float32(1.0), cs[:, 0]).astype(np.float32)
        xa = (x * gate[:, None]).astype(np.float32)
        bgt = np.where(cs == 1.0, bgt + 1.0, bgt)
        h1m = (h1m * DECAY * (1.0 - h1s) + xa @ W1.T + b1).astype(np.float32)
        h1s = (h1m > THRESH).astype(np.float32)
        h2m = (h2m * DECAY * (1.0 - h2s) + h1s @ W2.T + b2).astype(np.float32)
        h2s = (h2m > THRESH).astype(np.float32)
        c_in = np.concatenate([h1s, np.broadcast_to(freqs[t], (Bn, 4))], axis=1)
        cm = (cm * np.float32(0.1) * (1.0 - cs) + c_in @ Wc.T + bc).astype(np.float32)
        cs = (cm > THRESH).astype(np.float32)
        sum2 = sum2 + h2s
    rate = (sum2 / bgt).astype(np.float32)
    Xout = Xp.reshape(X_in.shape)
    return rate, Xout


def _assemble_X(inp, X_shape, idx):
    Xout = np.zeros(X_shape, np.float32)
    Xf = Xout.reshape(X_shape[0], DIN, T)
    clip = inp.reshape(X_shape[0], DIN, TAU_IN)
    for b in range(X_shape[0]):
        Xf[b, :, idx[b]:idx[b] + TAU_IN] = clip[b]
    return Xout


def _host_finish(H1Swin, idx, W2, b2, Wc, bc):
    """h2 / control / budget scans over all 300 steps, replicating the
    reference's fp32 op order exactly.  Returns (rate, gate_ok, cs_traj)."""
    t_arr = np.arange(T)
    freqs = np.stack([np.ones(T), (t_arr % 2 == 0), (t_arr % 10 == 0),
                      (t_arr % 100 == 0)], axis=1).astype(np.float32)
    Bn = H1Swin.shape[1]
    h2m = np.zeros((Bn, NCLS), np.float32); h2s = np.zeros((Bn, NCLS), np.float32)
    cm = np.zeros((Bn, 1), np.float32); cs = np.zeros((Bn, 1), np.float32)
    bgt = np.ones((Bn, 1), np.float32); sum2 = np.zeros((Bn, NCLS), np.float32)
    gate_ok = True
    for t in range(T):
        tau = t - idx
        rows = np.where((tau >= 0) & (tau < W))[0]
        h1s_t = np.zeros((Bn, HID), np.float32)
        if len(rows):
            h1s_t[rows] = H1Swin[tau[rows], rows]
        # gating assumption check: x is nonzero for rows with tau in [0,50);
        # the reference multiplies it by cs(t-1) (except t==0)
        if t > 0:
            xrows = rows[(tau[rows] < TAU_IN)]
            if len(xrows) and not np.all(cs[xrows, 0] == 1.0):
                gate_ok = False
        bgt = np.where(cs == 1.0, bgt + 1.0, bgt)
        h2m = (h2m * DECAY * (1.0 - h2s) + h1s_t @ W2.T + b2).astype(np.float32)
        h2s = (h2m > THRESH).astype(np.float32)
        c_in = np.concatenate([h1s_t, np.broadcast_to(freqs[t], (Bn, 4))], axis=1)
        cm = (cm * np.float32(0.1) * (1.0 - cs) + c_in @ Wc.T + bc).astype(np.float32)
        cs = (cm > THRESH).astype(np.float32)
        sum2 = sum2 + h2s
    rate = (sum2 / bgt).astype(np.float32)
    return rate, gate_ok


def kernel(**inputs):
    global LAST_RESULTS
    inp = np.ascontiguousarray(np.asarray(inputs["input"], dtype=np.float32))
    X_in = np.asarray(inputs["X"])
    idx = np.asarray(inputs["idx"]).astype(np.int64)
    W1 = np.ascontiguousarray(np.asarray(inputs["W1"], dtype=np.float32))
    b1 = np.asarray(inputs["b1"], dtype=np.float32)
    W2 = np.ascontiguousarray(np.asarray(inputs["W2"], dtype=np.float32))
    b2 = np.asarray(inputs["b2"], dtype=np.float32)
    Wc = np.ascontiguousarray(np.asarray(inputs["Wc"], dtype=np.float32))
    bc = np.asarray(inputs["bc"], dtype=np.float32)

    def fallback():
        rate, Xout = _np_exact_reference(
            inp, np.asarray(X_in, np.float32), idx, W1, b1, W2, b2, Wc, bc)
        return rate, Xout

    # -- structural preconditions for the fast device path --
    struct_ok = (
        inp.shape == (B, 2, 34, 34, TAU_IN)
        and X_in.shape == (B, 2, 34, 34, T)
        and idx.shape == (B,)
        and W1.shape == (HID, DIN) and W2.shape == (NCLS, HID)
        and Wc.shape == (1, HID + 4)
        and not np.any(X_in)
        and not np.any(b1)
        and idx.min() >= 0 and idx.max() <= T - TAU_IN
        and np.abs(inp).max() <= 1.0
        and float(np.abs(W1).sum(axis=1).max()) < 45.0  # no h1 spikes at tau>=51
    )
    if not struct_ok:
        return fallback()

    no_fb = os.environ.get("KERNEL_NO_FALLBACK") == "1"
    try:
        H1Swin = _run_device(inp, W1)
    except Exception:
        if no_fb:
            raise
        return fallback()

    rate, gate_ok = _host_finish(H1Swin, idx, W2, b2, Wc, bc)
    if not gate_ok:
        if no_fb:
            raise AssertionError("gate assumption violated")
        return fallback()

    Xout = _assemble_X(inp, X_in.shape, idx).reshape(X_in.shape)
    return rate, Xout


def _run_device(inp, W1):
    global LAST_RESULTS
    from concourse.bass_utils import run_bass_kernel_spmd

    clip = inp.reshape(B, DIN, TAU_IN)
    xT = np.zeros((DIN_PAD, TAU_IN, B), np.float32)
    xT[:DIN] = np.transpose(clip, (1, 2, 0))  # [DIN_PAD, tau, B]
    w1T = np.zeros((DIN_PAD, HID), np.float32)
    w1T[:DIN] = W1.T

    if MM_MODE == "fp32":
        in_maps = []
        for c in range(NCORES):
            xc = np.ascontiguousarray(
                xT[:, :, c * BLOC:(c + 1) * BLOC]).reshape(DIN_PAD,
                                                           TAU_IN * BLOC)
            in_maps.append({"x0": xc, "w0": w1T})
    else:
        whi = w1T.astype(np.float16)
        wlo = (w1T - whi.astype(np.float32)).astype(np.float16)
        in_maps = []
        for c in range(NCORES):
            xc = np.ascontiguousarray(
                xT[:, :, c * BLOC:(c + 1) * BLOC]).reshape(DIN_PAD,
                                                           TAU_IN * BLOC)
            xhi = xc.astype(np.float16)
            xlo = (xc - xhi.astype(np.float32)).astype(np.float16)
            in_maps.append({"xhi": xhi, "xlo": xlo, "whi": whi, "wlo": wlo})

    nc = _get_program(MM_MODE)
    trace = os.environ.get("KERNEL_TRACE") == "1"
    res = run_bass_kernel_spmd(nc, in_maps, core_ids=list(range(NCORES)),
                               trace=trace)
    LAST_RESULTS = res

    # decode: out[p, tau, j, b] -> H1S[tau, b, 128*j+p]
    parts = []
    for c in range(NCORES):
        arr = res.results[c]["h1s_out"]  # [128, W, JCH, BLOC]
        dec = np.transpose(arr, (1, 3, 2, 0)).reshape(W, BLOC, JCH * 128)
        parts.append(dec[:, :, :HID].astype(np.float32))
    return np.concatenate(parts, axis=1)  # [W, B, HID]


# revision 17
# speedup vs baseline: 1.4220x; 1.4220x over previous
"""Trainium2 Bass kernel for nn_FirstToSpike (spiking NN with sequential scan).

Structure of the computation (discovered by analysis of the reference):
  - X (the time-expanded input) is all zeros except a 50-step window per batch
    at t in [idx_b, idx_b+50) where the dense `input` clip is pasted.
  - b1 == 0, so before the window every batch's h1/h2 state is exactly zero;
    the layer-1 input at window step tau is Q[b, tau] = clip[b] @ W1.T,
    optionally gated by the control spike cs (a {0,1} scalar per batch).
  - The control unit's spike cs(t) is 1 whenever its membrane exceeds 0.5;
    with the given weight statistics it is 1 essentially always.  The device
    kernel assumes gate == 1 for all in-window steps; the host VERIFIES this
    afterwards from the returned spike trains and falls back to an exact
    numpy recomputation if it does not hold.
  - After the window, h1 membranes decay by 10x per step, so given
    max_h ||W1[h,:]||_1 < 45 (host-verified) no h1 spike can occur at
    tau >= 51.  Hence a 51-step aligned scan per batch captures every spike.

Device (8 cores, batch-parallel, 16 batches/core), per core:
  1. Q[h, (tau,b)] = W1.T-chunks @ x-chunks   (PE, fp32 or split-bf16)
  2. 52-step LIF scan (DVE):  h1m = fl(0.1*h1m)*(h1m_prev<=0.5) + Q[tau]
     spikes h1s = h1m > 0.5 recorded for all steps.
  3. DMA out the spike trains H1S [128, 52, 7, 16].

Host: assembles X output (exact paste), runs the (cheap) h2/control/bgt
scans over all 300 steps in fp32 exactly as the reference does, computes
rate = sum2/bgt, and verifies every structural assumption.
"""

import os
import numpy as np

B, T, DIN, HID, NCLS = 128, 300, 2312, 800, 10
DIN_PAD = 2432     # 19 * 128: zero-padded contraction dim (uniform k-chunks)
NCORES, BLOC = 8, 16
TAU_IN = 50        # input window length
W = 51             # aligned scan steps (spikes provably stop after tau=50
                   # given max_h ||W1[h,:]||_1 < 45, host-verified)
KCH = DIN_PAD // 128       # 19 uniform contraction chunks (tail zero-padded)
JCH = (HID + 127) // 128   # 7 output-row chunks (last has 32 rows)
NB = 2             # moving-dim blocks of 400 columns (25 tau x 16 b)
NBW = TAU_IN * BLOC // NB  # 400

DECAY = np.float32(0.1)
THRESH = np.float32(0.5)

# "fp32":   plain fp32 matmuls (walrus lowers each to 2 half-rate passes).
# "split16": x/w split into fp16 hi+lo, 3 passes (hi*hi + hi*lo + lo*hi) at
#            full PE rate; max |Q| error ~3e-6 (fp32-reorder level, verified).
MM_MODE = os.environ.get("K_MM_MODE", "split16")

# tau-blocks of the moving dimension: the scan consumes each block's Q while
# the next block's matmuls run on PE.  512 cols = one full PSUM bank; big
# blocks amortize per-matmul issue overhead, the small last block shortens
# the exposed scan tail.
NBS = [32, 18]

_prog_cache = {}
LAST_RESULTS = None  # BassKernelResults stash for test harness introspection


def _build_program(mm_mode):
    import concourse.bass as bass
    import concourse.tile as tile
    import concourse.mybir as mybir
    from concourse import bacc
    from concourse.bass import ds

    f32 = mybir.dt.float32
    bf16 = mybir.dt.bfloat16
    Alu = mybir.AluOpType

    # Bacc (not plain Bass): its compile() pipeline legalizes multi-wait
    # instructions (move_matmul_waits_to_ldweights / generate_event_semaphores)
    # which walrus codegen requires.
    nc = bacc.Bacc("TRN2")

    if mm_mode == "fp32":
        x_names, w_names, dt_in = ["x0"], ["w0"], f32
        passes = [(0, 0)]
    else:
        f16 = mybir.dt.float16
        x_names, w_names, dt_in = ["xhi", "xlo"], ["whi", "wlo"], f16
        passes = [(0, 0), (0, 1), (1, 0)]  # xhi*whi + xhi*wlo + xlo*whi

    x_dram = [nc.dram_tensor(n, [DIN_PAD, TAU_IN * BLOC], dt_in,
                             kind="ExternalInput") for n in x_names]
    w_dram = [nc.dram_tensor(n, [DIN_PAD, HID], dt_in, kind="ExternalInput")
              for n in w_names]
    u8 = mybir.dt.uint8
    h1s_out = nc.dram_tensor("h1s_out", [128, W, JCH, BLOC], u8,
                             kind="ExternalOutput")

    with tile.TileContext(nc) as tc:
        with (
            tc.tile_pool(name="big", bufs=1) as big,
            tc.tile_pool(name="state", bufs=1) as state,
            tc.tile_pool(name="psum", bufs=1, space="PSUM") as psum,
        ):
            x_sb = [big.tile([128, KCH, TAU_IN * BLOC], dt_in, tag=f"x{i}", name=f"x_sb{i}")
                    for i in range(len(x_dram))]
            w_sb = [big.tile([128, KCH, HID], dt_in, tag=f"w{i}", name=f"w_sb{i}")
                    for i in range(len(w_dram))]
            q_sb = big.tile([128, TAU_IN, JCH, BLOC], f32, tag="q")
            h1s_sb = big.tile([128, W, JCH, BLOC], u8, tag="h1s")

            h1m = state.tile([128, JCH, BLOC], f32, tag="h1m")
            zt = state.tile([128, JCH, BLOC], f32, tag="zt")   # zeros (fp32)
            ztm = state.tile([128, JCH, BLOC], u8, tag="ztm")  # zeros (mask)

            # j == JCH-1 h-rows beyond HID are never written by the matmul
            # copies; zero the whole Q buffer once so the scan sees clean pads.
            nc.gpsimd.memset(q_sb[:], 0.0)
            nc.vector.memset(h1m[:], 0.0)
            nc.vector.memset(zt[:], 0.0)
            nc.vector.memset(ztm[:], 0)

            # stream inputs interleaved per k-chunk, in the order the
            # matmul passes consume them (pass0 needs x[0]+w[0] first)
            loads, seen = [], set()
            for (xi, wi) in passes:
                if ("x", xi) not in seen:
                    seen.add(("x", xi)); loads.append((x_sb[xi], x_dram[xi]))
                if ("w", wi) not in seen:
                    seen.add(("w", wi)); loads.append((w_sb[wi], w_dram[wi]))
            for k in range(KCH):
                for t_sb, t_dram in loads[:2]:
                    nc.sync.dma_start(out=t_sb[:, k, :],
                                      in_=t_dram[128 * k:128 * (k + 1), :])
            for t_sb, t_dram in loads[2:]:
                for k in range(KCH):
                    nc.sync.dma_start(out=t_sb[:, k, :],
                                      in_=t_dram[128 * k:128 * (k + 1), :])

            def scan_step(tau):
                qv = q_sb[:, tau] if tau < TAU_IN else zt[:]
                mask = h1s_sb[:, tau - 1] if tau > 0 else ztm[:]
                # h1m' = fl(fl(0.1*h1m) + Q); where previous step spiked the
                # membrane resets, making h1m' exactly Q (reference rounding).
                nc.vector.scalar_tensor_tensor(
                    out=h1m[:], in0=h1m[:], scalar=0.1, in1=qv,
                    op0=Alu.mult, op1=Alu.add)
                nc.vector.copy_predicated(out=h1m[:], mask=mask, data=qv)
                nc.vector.tensor_scalar(h1s_sb[:, tau], h1m[:], 0.5, None,
                                        Alu.is_gt)

            assert sum(NBS) == TAU_IN
            n_acc = len(passes) * KCH
            lo = 0
            for nb, ntau in enumerate(NBS):
                hi = lo + ntau
                cols = ntau * BLOC
                pss = [psum.tile([128, cols], f32, tag=f"ps{j}",
                                 name=f"ps{j}") for j in range(JCH)]
                for pi, (xi, wi) in enumerate(passes):
                    for j in range(JCH):
                        mj = min(128, HID - 128 * j)
                        for k in range(KCH):
                            nc.tensor.matmul(
                                pss[j][:mj, :],
                                lhsT=w_sb[wi][:, k, ds(128 * j, mj)],
                                rhs=x_sb[xi][:, k, ds(lo * BLOC, cols)],
                                start=(pi == 0 and k == 0),
                                stop=(pi == len(passes) - 1 and k == KCH - 1))
                for j in range(JCH):
                    mj = min(128, HID - 128 * j)
                    # PSUM -> Q, reshaping (tau*16+b) columns into [tau, b]
                    nc.scalar.copy(
                        out=q_sb[:mj, ds(lo, ntau), j, :],
                        in_=pss[j][:mj, :].rearrange("p (t b) -> p t b",
                                                     b=BLOC))
                # consume this block's Q slices while the next block's
                # matmuls run on PE
                for tau in range(lo, hi):
                    scan_step(tau)
                if nb < len(NBS) - 1:
                    nc.sync.dma_start(out=h1s_out[:, lo:hi],
                                      in_=h1s_sb[:, lo:hi])
                lo = hi

            for tau in range(TAU_IN, W):
                scan_step(tau)
            last_lo = TAU_IN - NBS[-1]
            nc.sync.dma_start(out=h1s_out[:, last_lo:],
                              in_=h1s_sb[:, last_lo:])

    nc.finalize()  # Bacc.compile(): wait legalization + register allocation
    return nc


def _get_program(mm_mode):
    if mm_mode not in _prog_cache:
        _prog_cache[mm_mode] = _build_program(mm_mode)
    return _prog_cache[mm_mode]


def _np_exact_reference(inp, X_in, idx, W1, b1, W2, b2, Wc, bc):
    """Exact fp32 numpy replication of the reference (fallback path).

    Uses x_a @ W1.T == gate * (x @ W1.T) (exact: gate in {0,1}) when X has
    the zeros+window structure; otherwise fully dense (slow but correct).
    """
    Bn = X_in.shape[0]
    Xf = X_in.reshape(Bn, DIN, T)
    t_arr = np.arange(T)
    freqs = np.stack([np.ones(T), (t_arr % 2 == 0), (t_arr % 10 == 0),
                      (t_arr % 100 == 0)], axis=1).astype(np.float32)
    # paste
    Xp = Xf.copy()
    clip = inp.reshape(Bn, DIN, TAU_IN)
    for b in range(Bn):
        Xp[b, :, idx[b]:idx[b] + TAU_IN] = clip[b]
    h1m = np.zeros((Bn, HID), np.float32); h1s = np.zeros((Bn, HID), np.float32)
    h2m = np.zeros((Bn, NCLS), np.float32); h2s = np.zeros((Bn, NCLS), np.float32)
    cm = np.zeros((Bn, 1), np.float32); cs = np.zeros((Bn, 1), np.float32)
    bgt = np.ones((Bn, 1), np.float32); sum2 = np.zeros((Bn, NCLS), np.float32)
    for t in range(T):
        x = np.ascontiguousarray(Xp[:, :, t])
        gate = np.where(t == 0, np.float32(1.0), cs[:, 0]).astype(np.float32)
        xa = (x * gate[:, None]).astype(np.float32)
        bgt = np.where(cs == 1.0, bgt + 1.0, bgt)
        h1m = (h1m * DECAY * (1.0 - h1s) + xa @ W1.T + b1).astype(np.float32)
        h1s = (h1m > THRESH).astype(np.float32)
        h2m = (h2m * DECAY * (1.0 - h2s) + h1s @ W2.T + b2).astype(np.float32)
        h2s = (h2m > THRESH).astype(np.float32)
        c_in = np.concatenate([h1s, np.broadcast_to(freqs[t], (Bn, 4))], axis=1)
        cm = (cm * np.float32(0.1) * (1.0 - cs) + c_in @ Wc.T + bc).astype(np.float32)
        cs = (cm > THRESH).astype(np.float32)
        sum2 = sum2 + h2s
    rate = (sum2 / bgt).astype(np.float32)
    Xout = Xp.reshape(X_in.shape)
    return rate, Xout


def _assemble_X(inp, X_shape, idx):
    Xout = np.zeros(X_shape, np.float32)
    Xf = Xout.reshape(X_shape[0], DIN, T)
    clip = inp.reshape(X_shape[0], DIN, TAU_IN)
    for b in range(X_shape[0]):
        Xf[b, :, idx[b]:idx[b] + TAU_IN] = clip[b]
    return Xout


def _host_finish(H1Swin, idx, W2, b2, Wc, bc):
    """h2 / control / budget scans over all 300 steps, replicating the
    reference's fp32 op order exactly.  Returns (rate, gate_ok, cs_traj)."""
    t_arr = np.arange(T)
    freqs = np.stack([np.ones(T), (t_arr % 2 == 0), (t_arr % 10 == 0),
                      (t_arr % 100 == 0)], axis=1).astype(np.float32)
    Bn = H1Swin.shape[1]
    h2m = np.zeros((Bn, NCLS), np.float32); h2s = np.zeros((Bn, NCLS), np.float32)
    cm = np.zeros((Bn, 1), np.float32); cs = np.zeros((Bn, 1), np.float32)
    bgt = np.ones((Bn, 1), np.float32); sum2 = np.zeros((Bn, NCLS), np.float32)
    gate_ok = True
    for t in range(T):
        tau = t - idx
        rows = np.where((tau >= 0) & (tau < W))[0]
        h1s_t = np.zeros((Bn, HID), np.float32)
        if len(rows):
            h1s_t[rows] = H1Swin[tau[rows], rows]
        # gating assumption check: x is nonzero for rows with tau in [0,50);
        # the reference multiplies it by cs(t-1) (except t==0)
        if t > 0:
            xrows = rows[(tau[rows] < TAU_IN)]
            if len(xrows) and not np.all(cs[xrows, 0] == 1.0):
                gate_ok = False
        bgt = np.where(cs == 1.0, bgt + 1.0, bgt)
        h2m = (h2m * DECAY * (1.0 - h2s) + h1s_t @ W2.T + b2).astype(np.float32)
        h2s = (h2m > THRESH).astype(np.float32)
        c_in = np.concatenate([h1s_t, np.broadcast_to(freqs[t], (Bn, 4))], axis=1)
        cm = (cm * np.float32(0.1) * (1.0 - cs) + c_in @ Wc.T + bc).astype(np.float32)
        cs = (cm > THRESH).astype(np.float32)
        sum2 = sum2 + h2s
    rate = (sum2 / bgt).astype(np.float32)
    return rate, gate_ok


def kernel(**inputs):
    global LAST_RESULTS
    inp = np.ascontiguousarray(np.asarray(inputs["input"], dtype=np.float32))
    X_in = np.asarray(inputs["X"])
    idx = np.asarray(inputs["idx"]).astype(np.int64)
    W1 = np.ascontiguousarray(np.asarray(inputs["W1"], dtype=np.float32))
    b1 = np.asarray(inputs["b1"], dtype=np.float32)
    W2 = np.ascontiguousarray(np.asarray(inputs["W2"], dtype=np.float32))
    b2 = np.asarray(inputs["b2"], dtype=np.float32)
    Wc = np.ascontiguousarray(np.asarray(inputs["Wc"], dtype=np.float32))
    bc = np.asarray(inputs["bc"], dtype=np.float32)

    def fallback():
        rate, Xout = _np_exact_reference(
            inp, np.asarray(X_in, np.float32), idx, W1, b1, W2, b2, Wc, bc)
        return rate, Xout

    # -- structural preconditions for the fast device path --
    struct_ok = (
        inp.shape == (B, 2, 34, 34, TAU_IN)
        and X_in.shape == (B, 2, 34, 34, T)
        and idx.shape == (B,)
        and W1.shape == (HID, DIN) and W2.shape == (NCLS, HID)
        and Wc.shape == (1, HID + 4)
        and not np.any(X_in)
        and not np.any(b1)
        and idx.min() >= 0 and idx.max() <= T - TAU_IN
        and np.abs(inp).max() <= 1.0
        and float(np.abs(W1).sum(axis=1).max()) < 45.0  # no h1 spikes at tau>=51
    )
    if not struct_ok:
        return fallback()

    no_fb = os.environ.get("KERNEL_NO_FALLBACK") == "1"
    try:
        H1Swin = _run_device(inp, W1)
    except Exception:
        if no_fb:
            raise
        return fallback()

    rate, gate_ok = _host_finish(H1Swin, idx, W2, b2, Wc, bc)
    if not gate_ok:
        if no_fb:
            raise AssertionError("gate assumption violated")
        return fallback()

    Xout = _assemble_X(inp, X_in.shape, idx).reshape(X_in.shape)
    return rate, Xout


def _run_device(inp, W1):
    global LAST_RESULTS
    from concourse.bass_utils import run_bass_kernel_spmd

    clip = inp.reshape(B, DIN, TAU_IN)
    xT = np.zeros((DIN_PAD, TAU_IN, B), np.float32)
    xT[:DIN] = np.transpose(clip, (1, 2, 0))  # [DIN_PAD, tau, B]
    w1T = np.zeros((DIN_PAD, HID), np.float32)
    w1T[:DIN] = W1.T

    if MM_MODE == "fp32":
        in_maps = []
        for c in range(NCORES):
            xc = np.ascontiguousarray(
                xT[:, :, c * BLOC:(c + 1) * BLOC]).reshape(DIN_PAD,
                                                           TAU_IN * BLOC)
            in_maps.append({"x0": xc, "w0": w1T})
    else:
        whi = w1T.astype(np.float16)
        wlo = (w1T - whi.astype(np.float32)).astype(np.float16)
        in_maps = []
        for c in range(NCORES):
            xc = np.ascontiguousarray(
                xT[:, :, c * BLOC:(c + 1) * BLOC]).reshape(DIN_PAD,
                                                           TAU_IN * BLOC)
            xhi = xc.astype(np.float16)
            xlo = (xc - xhi.astype(np.float32)).astype(np.float16)
            in_maps.append({"xhi": xhi, "xlo": xlo, "whi": whi, "wlo": wlo})

    nc = _get_program(MM_MODE)
    trace = os.environ.get("KERNEL_TRACE") == "1"
    res = run_bass_kernel_spmd(nc, in_maps, core_ids=list(range(NCORES)),
                               trace=trace)
    LAST_RESULTS = res

    # decode: out[p, tau, j, b] -> H1S[tau, b, 128*j+p]
    parts = []
    for c in range(NCORES):
        arr = res.results[c]["h1s_out"]  # [128, W, JCH, BLOC]
        dec = np.transpose(arr, (1, 3, 2, 0)).reshape(W, BLOC, JCH * 128)
        parts.append(dec[:, :, :HID].astype(np.float32))
    return np.concatenate(parts, axis=1)  # [W, B, HID]
